# revision 50
# baseline (speedup 1.0000x reference)
"""Multi-head attention Trainium2 kernel, 8-way sharded.  ~277 us HW.

Problem: x[4,2048,1024] -> qkv proj (w_qkv [3072,1024]) -> 16-head attention
with key-padding mask -> tail proj (w_tail [1024,1024]) + b_tail.

Sharding: 8 shards = 4 batches x 2 head-groups (8 heads each = 4 head PAIRS).
Host unshards: out[b] = y_part[2b] + y_part[2b+1] + b_tail.  No collectives.

Key ideas (vs 603 us v1 baseline):
- Host-side key gather: masked keys (~50%) are dropped before the kernel;
  only ceil(max_kept/128) key blocks (9 here) are computed instead of 16.
  Mask correctness comes from zeroing V rows and the denominator-ones
  column for pad keys, so exp needs no bias at all.
- bf16 matmul operands everywhere (fp32r matmuls self-load weights serially,
  ~175 ns each; bf16 gets separate LDWEIGHTS that the PE reorder window
  hides under the previous matmul's stream).
- ST row-tiling: the two heads of a pair have K=64 contractions, placed in
  array rows 0-63 / 64-127 via tile_position -> both score matmuls run
  concurrently.
- One exp per (pair, kb, qb): N=1024 covering both heads' [128 keys x 512 q]
  scores in a 2-bank PSUM tile.
- Normalization fully off the critical path: attn^T and the denominator row
  are staged to SBUF per unit; denominator rows of a whole qb round are
  gathered (SBUF->SBUF DMA) into one [8,512] tile, reciprocal'd ONCE
  (DVE reciprocal is ~6.5 ns/elem - batching is essential), broadcast with
  a tiny selector matmul, and applied during the next round's slack.
  The reciprocal itself is a deque quantum consumed ~11 us into the next
  round: emitting it at round end blocks the DVE right when the aux-psum
  ring needs prompt evacuations (measured +12 us).
- Single software-pipelined emission stream with explicit deadlines:
  projection / normalize / tail quanta are injected between attention steps
  so the PE fills the slack under the Act(exp) stream; phases overlap.
  PSUM budget: score ring 2x2 banks + 2 AV accumulators + 2 aux = 8 banks.
"""

import time as _time

import numpy as np
import ml_dtypes
from contextlib import ExitStack

import concourse.bass as bass
import concourse.mybir as mybir
import concourse.tile as tile
from concourse.bass_utils import run_bass_kernel_spmd

# ---------------------------------------------------------------------------
# walrus in this env accepts at most 2 sync waits per instruction; Tile's
# scheduler emits up to 10. Post-pass: peel excess waits onto same-engine
# NoOps inserted immediately before the offending instruction (same engine
# stream position => identical synchronization semantics).
MAX_WAITS = 1


def split_excess_waits(nc):
    for fn in nc.m.functions:
        for bb in fn.blocks:
            insts = list(bb.instructions)
            out = []
            changed = False
            for inst in insts:
                si = inst.sync_info
                waits = list(si.on_wait) if si is not None else []
                if len(waits) > MAX_WAITS:
                    extra = waits[:-MAX_WAITS]
                    for ci in range(0, len(extra), MAX_WAITS):
                        chunk = extra[ci:ci + MAX_WAITS]
                        nop = mybir.InstNoOp(
                            name=f"{inst.name}-ws{ci}", ins=[], outs=[])
                        nop.engine = inst.engine
                        nop.sync_info = mybir.SyncInfo(
                            on_wait=chunk, on_update=[])
                        out.append(nop)
                    inst.sync_info = mybir.SyncInfo(
                        on_wait=waits[-MAX_WAITS:],
                        on_update=list(si.on_update))
                    changed = True
                out.append(inst)
            if changed:
                bb.instructions = out
# ---------------------------------------------------------------------------

D_MODEL = 1024
N_HEAD = 16
D_HEAD = 64
BN, T = 4, 2048
HPC = 8                      # heads per core
NPAIR = HPC // 2             # 4 head pairs per core
CAT = HPC * D_HEAD           # 512 per-core tail contraction
KC = D_MODEL // 128          # 8 contraction chunks
NQB = T // 512               # 4 q blocks of 512
F32 = mybir.dt.float32
F32R = mybir.dt.float32r
BF16 = mybir.dt.bfloat16
BF = ml_dtypes.bfloat16


def build_nc(nkb, split_waits=True):
    """nkb: number of 128-key blocks of gathered (kept+pad) keys."""
    KT = nkb * 128
    NKC = (KT + 511) // 512      # kproj 512-col chunks
    nc = bass.Bass()
    xqT = nc.declare_dram_parameter("xqT", [D_MODEL, T], BF16, isOutput=False)
    xkT = nc.declare_dram_parameter("xkT", [D_MODEL, KT], BF16, isOutput=False)
    wqT = nc.declare_dram_parameter("wqT", [D_MODEL, NPAIR * 128], BF16, isOutput=False)
    wkT = nc.declare_dram_parameter("wkT", [D_MODEL, NPAIR * 128], BF16, isOutput=False)
    wvT = nc.declare_dram_parameter("wvT", [D_MODEL, CAT], BF16, isOutput=False)
    wtailT = nc.declare_dram_parameter("wtailT", [CAT, D_MODEL], BF16, isOutput=False)
    keep = nc.declare_dram_parameter("keep", [KT], F32, isOutput=False)
    ones8 = nc.declare_dram_parameter("ones8", [128, HPC], BF16, isOutput=False)
    sel4 = nc.declare_dram_parameter("sel4", [HPC, NPAIR * 128], F32R, isOutput=False)
    y = nc.declare_dram_parameter("y", [T, D_MODEL], F32, isOutput=True)

    with ExitStack() as ctx:
        tc = ctx.enter_context(tile.TileContext(nc))

        const = ctx.enter_context(tc.tile_pool(name="const", bufs=1))
        xq_pool = ctx.enter_context(tc.tile_pool(name="xq", bufs=1))
        xk_pool = ctx.enter_context(tc.tile_pool(name="xk", bufs=1))
        qk_pool = ctx.enter_context(tc.tile_pool(name="qk", bufs=1))
        va_pool = ctx.enter_context(tc.tile_pool(name="va", bufs=1))
        num_pool = ctx.enter_context(tc.tile_pool(name="num", bufs=1))
        w_pool = ctx.enter_context(tc.tile_pool(name="w", bufs=1))
        p_pool = ctx.enter_context(tc.tile_pool(name="p", bufs=1))
        stag_pool = ctx.enter_context(tc.tile_pool(name="stag", bufs=1))
        rb_pool = ctx.enter_context(tc.tile_pool(name="rb", bufs=1))
        ysb_pool = ctx.enter_context(tc.tile_pool(name="ysb", bufs=1))
        stp_pool = ctx.enter_context(tc.tile_pool(name="stp", bufs=1, space="PSUM"))
        avp_pool = ctx.enter_context(tc.tile_pool(name="avp", bufs=1, space="PSUM"))
        aux_pool = ctx.enter_context(tc.tile_pool(name="aux", bufs=1, space="PSUM"))

        # ---- consts
        sel4t = const.tile([HPC, NPAIR, 128], F32R, name="sel4")
        nc.sync.dma_start(
            out=sel4t, in_=sel4.rearrange("p (j c) -> p j c", c=128))
        keepc = const.tile([128, nkb], F32, name="keep")
        nc.sync.dma_start(out=keepc, in_=keep.rearrange("(j p) -> p j", p=128))
        ones8t = const.tile([128, HPC], BF16, name="ones8")
        nc.sync.dma_start(out=ones8t, in_=ones8[:, :])

        # ---- inputs (order = DMA priority: kproj/qproj first wave)
        wks = [w_pool.tile([128, KC, 128], BF16, tag=f"wk{j}", name=f"wk{j}")
               for j in range(NPAIR)]
        wqs = [w_pool.tile([128, KC, 128], BF16, tag=f"wq{j}", name=f"wq{j}")
               for j in range(NPAIR)]
        xks = [xk_pool.tile([128, KT], BF16, tag=f"xk{kc}", name=f"xk{kc}")
               for kc in range(KC)]
        xqs = [xq_pool.tile([128, T], BF16, tag=f"xq{kc}", name=f"xq{kc}")
               for kc in range(KC)]
        nc.sync.dma_start(
            out=wks[0],
            in_=wkT.rearrange("(kc p) c -> p kc c", p=128)[:, :, 0:128])
        for kc in range(KC):
            nc.sync.dma_start(out=xks[kc][:, 0:512],
                              in_=xkT[kc * 128:(kc + 1) * 128, 0:512])
        nc.sync.dma_start(
            out=wqs[0],
            in_=wqT.rearrange("(kc p) c -> p kc c", p=128)[:, :, 0:128])
        for kc in range(KC):
            nc.sync.dma_start(out=xqs[kc][:, 0:512],
                              in_=xqT[kc * 128:(kc + 1) * 128, 0:512])
        wvs = [w_pool.tile([128, CAT], BF16, tag=f"wv{kc}", name=f"wv{kc}")
               for kc in range(KC)]
        for kc in range(KC):
            nc.sync.dma_start(out=wvs[kc], in_=wvT[kc * 128:(kc + 1) * 128, :])
        if KT > 512:
            for kc in range(KC):
                nc.sync.dma_start(out=xks[kc][:, 512:KT],
                                  in_=xkT[kc * 128:(kc + 1) * 128, 512:KT])
        for j in range(1, NPAIR):
            nc.sync.dma_start(
                out=wks[j],
                in_=wkT.rearrange("(kc p) c -> p kc c", p=128)[
                    :, :, j * 128:(j + 1) * 128])
            nc.sync.dma_start(
                out=wqs[j],
                in_=wqT.rearrange("(kc p) c -> p kc c", p=128)[
                    :, :, j * 128:(j + 1) * 128])
        for kc in range(KC):
            nc.sync.dma_start(out=xqs[kc][:, 512:T],
                              in_=xqT[kc * 128:(kc + 1) * 128, 512:T])
        wts = [w_pool.tile([128, D_MODEL], BF16, tag=f"wt{c}", name=f"wt{c}")
               for c in range(CAT // 128)]
        for c in range(CAT // 128):
            nc.sync.dma_start(out=wts[c], in_=wtailT[c * 128:(c + 1) * 128, :])

        # ---- persistent intermediates
        qts = [qk_pool.tile([128, T], BF16, tag=f"qt{j}", name=f"qt{j}")
               for j in range(NPAIR)]
        kts = [qk_pool.tile([128, KT], BF16, tag=f"kt{j}", name=f"kt{j}")
               for j in range(NPAIR)]
        vaugs = [va_pool.tile([128, HPC, D_HEAD + 1], BF16, tag=f"va{t}",
                              name=f"va{t}") for t in range(nkb)]
        nums = [num_pool.tile([128, T], BF16, tag=f"nm{j}", name=f"nm{j}")
                for j in range(NPAIR)]
        # per-pair staged attn^T (+denominator row); written each round,
        # read by the deferred normalize during the NEXT round => bufs=2.
        def stag(j, h):
            return stag_pool.tile([D_HEAD + 1, 512], F32, tag=f"sg{j}{h}",
                                  name=f"sg{j}{h}", bufs=2)

        # ---- work quanta (each: one aux-psum accumulation group + evac)
        def vproj(tb):
            vp = aux_pool.tile([128, 512], F32, tag="aux", bufs=2)
            for kc in range(KC):
                nc.tensor.matmul(vp, xks[kc][:, tb * 128:(tb + 1) * 128],
                                 wvs[kc], start=(kc == 0), stop=(kc == KC - 1))
            va = vaugs[tb]
            nc.vector.tensor_scalar_mul(
                va[:, :, 0:D_HEAD],
                vp.rearrange("p (h d) -> p h d", h=HPC),
                keepc[:, tb:tb + 1])
            nc.vector.tensor_scalar_mul(
                va[:, :, D_HEAD:D_HEAD + 1].rearrange("p h o -> p (h o)"),
                ones8t,
                keepc[:, tb:tb + 1])

        def kproj_range(j, n0, n1):
            kp = aux_pool.tile([128, 512], F32, tag="aux", bufs=2)
            for kc in range(KC):
                nc.tensor.matmul(kp[:, 0:n1 - n0], wks[j][:, kc, :],
                                 xks[kc][:, n0:n1],
                                 start=(kc == 0), stop=(kc == KC - 1))
            nc.vector.tensor_copy(out=kts[j][:, n0:n1], in_=kp[:, 0:n1 - n0])

        def kproj(j, c):
            kproj_range(j, c * 512, min(c * 512 + 512, KT))

        def qproj(j, n):
            n0 = n * 512
            qp = aux_pool.tile([128, 512], F32, tag="aux", bufs=2)
            for kc in range(KC):
                nc.tensor.matmul(qp, wqs[j][:, kc, :], xqs[kc][:, n0:n0 + 512],
                                 start=(kc == 0), stop=(kc == KC - 1))
            nc.vector.tensor_copy(out=qts[j][:, n0:n0 + 512], in_=qp)

        def tailq(tb, n, evac_scalar=False):
            n0 = n * 512
            yp = aux_pool.tile([128, 512], F32, tag="aux", bufs=2)
            for c in range(NPAIR):
                nc.tensor.matmul(yp, nums[c][:, tb * 128:(tb + 1) * 128],
                                 wts[c][:, n0:n0 + 512],
                                 start=(c == 0), stop=(c == NPAIR - 1))
            y_sb = ysb_pool.tile([128, 512], F32, tag="ys", bufs=2)
            if evac_scalar:
                # drain only: the Scalar engine is idle there and this keeps
                # the aux-ring turnover off the DVE queue (which carries the
                # final reciprocal)
                nc.scalar.copy(out=y_sb, in_=yp)
            else:
                nc.vector.tensor_copy(out=y_sb, in_=yp)
            nc.sync.dma_start(out=y[tb * 128:(tb + 1) * 128, n0:n0 + 512],
                              in_=y_sb)

        # ---- deferred normalization for one qb round. The reciprocal (DVE,
        # ~6.5 ns/elem, batched to one [8,512] op/round) is emitted at round
        # end so it runs during the next round's attention; the broadcast +
        # apply quanta are consumed from the deque well after it finished.
        def make_norm_quanta(qb, rball, round_stags):
            q0 = qb * 512

            def recipq():
                rtmp = rb_pool.tile([HPC, 512], F32, tag="rtmp", bufs=2)
                nc.vector.reciprocal(out=rtmp, in_=rball)
                rrec = rb_pool.tile([HPC, 512], F32R, tag="rrec", bufs=2)
                nc.vector.tensor_copy(out=rrec, in_=rtmp)
                rrec_ref[0] = rrec

            rrec_ref = [None]
            quanta = [recipq]

            def normj(j):
                rbp = aux_pool.tile([128, 512], F32, tag="aux", bufs=2)
                nc.tensor.matmul(rbp, sel4t[:, j, :], rrec_ref[0],
                                 start=True, stop=True)
                nc.vector.tensor_tensor(
                    out=nums[j][0:64, q0:q0 + 512],
                    in0=round_stags[j][0][0:64, :],
                    in1=rbp[0:64, :], op=mybir.AluOpType.mult)
                nc.vector.tensor_tensor(
                    out=nums[j][64:128, q0:q0 + 512],
                    in0=round_stags[j][1][0:64, :],
                    in1=rbp[64:128, :], op=mybir.AluOpType.mult)

            quanta.extend(lambda jj=j: normj(jj) for j in range(NPAIR))
            return quanta

        # general work deque (FIFO; consumed one per slot)
        deque = []
        for n in range(1, NQB):
            for j in range(NPAIR):
                deque.append((lambda jj=j, nn=n: qproj(jj, nn)))

        def pop_deque():
            if deque:
                deque.pop(0)()
            else:
                # deque dry => this stretch is Act-bound; keep the PE's
                # activity monitor fed so the clock gate never re-throttles
                # (a ~3.4us low-duty window halves the PE clock)
                dmy = aux_pool.tile([128, 512], F32, tag="aux", bufs=2)
                for i in range(2):
                    nc.tensor.matmul(dmy, qts[0][0:64, 0:128],
                                     qts[0][0:64, 0:512],
                                     start=(i == 0), stop=(i == 1))

        # direct normalize for one pair (used in the hand-scheduled drain)
        def normd(j, q0, selap, rrec_ap, sgA, sgB):
            rbp = aux_pool.tile([128, 512], F32, tag="aux", bufs=2)
            nc.tensor.matmul(rbp, selap, rrec_ap, start=True, stop=True)
            nc.vector.tensor_tensor(
                out=nums[j][0:64, q0:q0 + 512], in0=sgA[0:64, :],
                in1=rbp[0:64, :], op=mybir.AluOpType.mult)
            nc.vector.tensor_tensor(
                out=nums[j][64:128, q0:q0 + 512], in0=sgB[0:64, :],
                in1=rbp[64:128, :], op=mybir.AluOpType.mult)

        # ---- attention unit
        def unit(u, j, qb, rball, rballB=None):
            q0 = qb * 512
            h0, h1 = 2 * j, 2 * j + 1
            avpA = avp_pool.tile([D_HEAD + 1, 512], F32, tag="avpA",
                                 name="avpA")
            avpB = avp_pool.tile([D_HEAD + 1, 512], F32, tag="avpB",
                                 name="avpB")
            ps = {}

            def st_exp(kb):
                stp = stp_pool.tile([128, 2, 512], F32, tag="stp", bufs=2)
                nc.tensor.matmul(
                    stp[:, 0, :], kts[j][0:64, kb * 128:(kb + 1) * 128],
                    qts[j][0:64, q0:q0 + 512], start=True, stop=True,
                    tile_position=(0, 0))
                nc.tensor.matmul(
                    stp[:, 1, :], kts[j][64:128, kb * 128:(kb + 1) * 128],
                    qts[j][64:128, q0:q0 + 512], start=True, stop=True,
                    tile_position=(64, 0))
                p = p_pool.tile([128, 2, 512], BF16, tag="p", bufs=3)
                nc.scalar.activation(
                    out=p, in_=stp, func=mybir.ActivationFunctionType.Exp,
                    scale=0.125)
                ps[kb] = p

            def av(kb):
                p = ps.pop(kb)
                nc.tensor.matmul(avpA, vaugs[kb][:, h0, :], p[:, 0, :],
                                 start=(kb == 0), stop=(kb == nkb - 1))
                nc.tensor.matmul(avpB, vaugs[kb][:, h1, :], p[:, 1, :],
                                 start=(kb == 0), stop=(kb == nkb - 1))

            st_exp(0)
            for kb in range(1, nkb):
                # scheduled filler work for this slot
                if u == 0:
                    if kb == 1:
                        vproj(0)
                        vproj(1)
                        vproj(2)
                    elif kb <= NKC:
                        kproj(0, kb - 1)
                        if kb + 1 < nkb:
                            vproj(kb + 1)
                    elif kb + 1 < nkb:
                        vproj(kb + 1)
                elif u in (1, 2, 3):
                    if 1 <= kb < NKC:
                        kproj(j, kb)
                    elif kb >= 3:
                        pop_deque()
                else:
                    pop_deque()
                st_exp(kb)
                av(kb - 1)
            av(nkb - 1)

            # stage attn^T + denominator row; gather D rows for the round
            sgA, sgB = stag(j, 0), stag(j, 1)
            nc.vector.tensor_copy(out=sgA, in_=avpA)
            nc.vector.tensor_copy(out=sgB, in_=avpB)
            if rballB is not None:
                # last pair of the last round: its own 2-row gather so the
                # drain reciprocal is partition-base-0 (32-align rule)
                nc.sync.dma_start(out=rballB[0:1, :],
                                  in_=sgA[D_HEAD:D_HEAD + 1, :])
                nc.sync.dma_start(out=rballB[1:2, :],
                                  in_=sgB[D_HEAD:D_HEAD + 1, :])
            else:
                nc.sync.dma_start(out=rball[h0:h0 + 1, :],
                                  in_=sgA[D_HEAD:D_HEAD + 1, :])
                nc.sync.dma_start(out=rball[h1:h1 + 1, :],
                                  in_=sgB[D_HEAD:D_HEAD + 1, :])
            return sgA, sgB

        # ---- prologue: minimum to start unit(p0, qb0)
        kproj(0, 0)
        qproj(0, 0)

        u = 0
        for qb in range(NQB):
            last = qb == NQB - 1
            rball = rb_pool.tile([HPC, 512], F32, tag="rball", bufs=2)
            rballB = (rb_pool.tile([2, 512], F32, tag="rballB", name="rballB")
                      if last else None)
            rrecA = None
            round_stags = []
            for j in range(NPAIR):
                if u in (1, 2, 3):
                    kproj(j, 0)
                    qproj(j, 0)
                round_stags.append(
                    unit(u, j, qb, rball,
                         rballB if (last and j == NPAIR - 1) else None))
                u += 1
                if last and j == NPAIR - 2:
                    # pairs 0-2 denominators are complete: their reciprocal
                    # runs on the idle DVE under the last unit's attention,
                    # leaving only a [2,512] recip on the drain path.
                    rtmpA = rb_pool.tile([6, 512], F32, tag="rtA", name="rtA")
                    nc.vector.reciprocal(out=rtmpA, in_=rball[0:6, :])
                    rrecA = rb_pool.tile([6, 512], F32R, tag="rrA", name="rrA")
                    nc.vector.tensor_copy(out=rrecA, in_=rtmpA)
            if not last:
                # tail(qb-1) first (dense, dependency-free at round start),
                # then recip+norm(qb): the reciprocal lands ~11us into the
                # next round where the DVE is idle, before its consumers.
                if qb >= 1:
                    for tb in range((qb - 1) * 4, qb * 4):
                        for n in range(2):
                            deque.append((lambda t=tb, nn=n: tailq(t, nn)))
                deque.extend(make_norm_quanta(qb, rball, round_stags))
        # ---- hand-scheduled drain for the last round
        qb = NQB - 1
        q0 = qb * 512
        # pair-3 reciprocal first in the drain's DVE stream: tail evacs go
        # through the Scalar engine, so nothing queues behind it
        rtmpB = rb_pool.tile([2, 512], F32, tag="rtB", name="rtB")
        nc.vector.reciprocal(out=rtmpB, in_=rballB)
        rrecB = rb_pool.tile([2, 512], F32R, tag="rrB", name="rrB")
        nc.vector.tensor_copy(out=rrecB, in_=rtmpB)
        for tb in range((qb - 1) * 4, qb * 4):
            for n in range(2):
                deque.append((lambda t=tb, nn=n: tailq(t, nn, True)))
        pop_deque()
        pop_deque()
        pop_deque()
        for j in range(NPAIR - 1):
            normd(j, q0, sel4t[0:6, j, :], rrecA,
                  round_stags[j][0], round_stags[j][1])
        # the remaining tail(qb-1) work covers the pair-3 reciprocal latency
        for _ in range(5):
            pop_deque()
        # pair 3: selector rows 0-1 of pair 0 have exactly the 2-row pattern
        normd(NPAIR - 1, q0, sel4t[0:2, 0, :], rrecB,
              round_stags[NPAIR - 1][0], round_stags[NPAIR - 1][1])
        for tb in range(qb * 4, (qb + 1) * 4):
            for n in range(2):
                deque.append((lambda t=tb, nn=n: tailq(t, nn, True)))
        while deque:
            pop_deque()

    if split_waits:
        split_excess_waits(nc)
    return nc


_NC_CACHE = {}


def _get_nc(nkb):
    if nkb not in _NC_CACHE:
        _NC_CACHE[nkb] = build_nc(nkb)
    return _NC_CACHE[nkb]


def make_in_maps(x, mask, w_qkv, w_tail):
    """Shard full inputs into 8 per-core input maps (with key gather)."""
    x = np.asarray(x, dtype=np.float32)
    mask = np.asarray(mask, dtype=np.int32)
    w_qkv = np.asarray(w_qkv, dtype=np.float32)
    w_tail = np.asarray(w_tail, dtype=np.float32)

    # per-batch kept-key gather
    idxs = [np.nonzero(mask[b] != 0)[0] for b in range(BN)]
    nkb = max(4, max((len(ix) + 127) // 128 for ix in idxs))
    KT = nkb * 128

    xk_all, keep_all = [], []
    for b in range(BN):
        ix = idxs[b]
        m = len(ix)
        xk = np.zeros((KT, D_MODEL), dtype=np.float32)
        xk[:m] = x[b][ix]
        kp = np.zeros((KT,), dtype=np.float32)
        kp[:m] = 1.0
        xk_all.append(xk)
        keep_all.append(kp)

    w3 = w_qkv.reshape(N_HEAD, 3, D_HEAD, D_MODEL)  # [head, q|k|v, d, dm]
    # sel4[:, j*128:(j+1)*128]: broadcast selector for pair j
    selv = np.zeros((HPC, NPAIR * 128), np.float32)
    for j in range(NPAIR):
        selv[2 * j, j * 128:j * 128 + 64] = 1.0
        selv[2 * j + 1, j * 128 + 64:(j + 1) * 128] = 1.0

    in_maps = []
    for c in range(8):
        b, hg = c // 2, c % 2
        heads = [hg * HPC + i for i in range(HPC)]
        wq = np.concatenate([w3[h, 0] for h in heads], axis=0)  # [512, 1024]
        wk = np.concatenate([w3[h, 1] for h in heads], axis=0)
        wv = np.concatenate([w3[h, 2] for h in heads], axis=0)
        wt = w_tail[:, hg * CAT:(hg + 1) * CAT]  # [1024, 512]
        in_maps.append({
            "xqT": np.ascontiguousarray(x[b].T).astype(BF),
            "xkT": np.ascontiguousarray(xk_all[b].T).astype(BF),
            "wqT": np.ascontiguousarray(wq.T).astype(BF),
            "wkT": np.ascontiguousarray(wk.T).astype(BF),
            "wvT": np.ascontiguousarray(wv.T).astype(BF),
            "wtailT": np.ascontiguousarray(wt.T).astype(BF),
            "keep": keep_all[b],
            "ones8": np.ones((128, HPC), dtype=BF),
            "sel4": selv,
        })
    return in_maps, nkb


def kernel(x, mask, w_qkv, w_tail, b_tail):
    in_maps, nkb = make_in_maps(x, mask, w_qkv, w_tail)
    nc = _get_nc(nkb)
    last_err = None
    for _attempt in range(3):
        try:
            res = run_bass_kernel_spmd(nc, in_maps, list(range(8))).results
            break
        except Exception as e:  # transient device/runtime errors: retry
            last_err = e
            _time.sleep(3.0)
    else:
        raise last_err
    out = np.empty((BN, T, D_MODEL), dtype=np.float32)
    b_tail = np.asarray(b_tail, dtype=np.float32)
    for b in range(BN):
        out[b] = res[2 * b]["y"] + res[2 * b + 1]["y"] + b_tail
    return out


# revision 51
# speedup vs baseline: 1.1514x; 1.1514x over previous
"""Multi-head attention Trainium2 kernel, 8-way sharded.  ~277 us HW.

Problem: x[4,2048,1024] -> qkv proj (w_qkv [3072,1024]) -> 16-head attention
with key-padding mask -> tail proj (w_tail [1024,1024]) + b_tail.

Sharding: 8 shards = 4 batches x 2 head-groups (8 heads each = 4 head PAIRS).
Host unshards: out[b] = y_part[2b] + y_part[2b+1] + b_tail.  No collectives.

Key ideas (vs 603 us v1 baseline):
- Host-side key gather: masked keys (~50%) are dropped before the kernel;
  only ceil(max_kept/128) key blocks (9 here) are computed instead of 16.
  Mask correctness comes from zeroing V rows and the denominator-ones
  column for pad keys, so exp needs no bias at all.
- bf16 matmul operands everywhere (fp32r matmuls self-load weights serially,
  ~175 ns each; bf16 gets separate LDWEIGHTS that the PE reorder window
  hides under the previous matmul's stream).
- ST row-tiling: the two heads of a pair have K=64 contractions, placed in
  array rows 0-63 / 64-127 via tile_position -> both score matmuls run
  concurrently.
- One exp per (pair, kb, qb): N=1024 covering both heads' [128 keys x 512 q]
  scores in a 2-bank PSUM tile.
- Normalization fully off the critical path: attn^T and the denominator row
  are staged to SBUF per unit; denominator rows of a whole qb round are
  gathered (SBUF->SBUF DMA) into one [8,512] tile, reciprocal'd ONCE
  (DVE reciprocal is ~6.5 ns/elem - batching is essential), broadcast with
  a tiny selector matmul, and applied during the next round's slack.
  The reciprocal itself is a deque quantum consumed ~11 us into the next
  round: emitting it at round end blocks the DVE right when the aux-psum
  ring needs prompt evacuations (measured +12 us).
- Single software-pipelined emission stream with explicit deadlines:
  projection / normalize / tail quanta are injected between attention steps
  so the PE fills the slack under the Act(exp) stream; phases overlap.
  PSUM budget: score ring 2x2 banks + 2 AV accumulators + 2 aux = 8 banks.
"""

import time as _time

import numpy as np
import ml_dtypes
from contextlib import ExitStack

import concourse.bass as bass
import concourse.mybir as mybir
import concourse.tile as tile
from concourse.bass_utils import run_bass_kernel_spmd

# ---------------------------------------------------------------------------
# walrus in this env accepts at most 2 sync waits per instruction; Tile's
# scheduler emits up to 10. Post-pass: peel excess waits onto same-engine
# NoOps inserted immediately before the offending instruction (same engine
# stream position => identical synchronization semantics).
MAX_WAITS = 1


def split_excess_waits(nc):
    for fn in nc.m.functions:
        for bb in fn.blocks:
            insts = list(bb.instructions)
            out = []
            changed = False
            for inst in insts:
                si = inst.sync_info
                waits = list(si.on_wait) if si is not None else []
                if len(waits) > MAX_WAITS:
                    extra = waits[:-MAX_WAITS]
                    for ci in range(0, len(extra), MAX_WAITS):
                        chunk = extra[ci:ci + MAX_WAITS]
                        nop = mybir.InstNoOp(
                            name=f"{inst.name}-ws{ci}", ins=[], outs=[])
                        nop.engine = inst.engine
                        nop.sync_info = mybir.SyncInfo(
                            on_wait=chunk, on_update=[])
                        out.append(nop)
                    inst.sync_info = mybir.SyncInfo(
                        on_wait=waits[-MAX_WAITS:],
                        on_update=list(si.on_update))
                    changed = True
                out.append(inst)
            if changed:
                bb.instructions = out
# ---------------------------------------------------------------------------

D_MODEL = 1024
N_HEAD = 16
D_HEAD = 64
BN, T = 4, 2048
HPC = 8                      # heads per core
NPAIR = HPC // 2             # 4 head pairs per core
CAT = HPC * D_HEAD           # 512 per-core tail contraction
KC = D_MODEL // 128          # 8 contraction chunks
NQB = T // 512               # 4 q blocks of 512
F32 = mybir.dt.float32
F32R = mybir.dt.float32r
BF16 = mybir.dt.bfloat16
BF = ml_dtypes.bfloat16


def build_nc(nkb, split_waits=True):
    """nkb: number of 128-key blocks of gathered (kept+pad) keys."""
    KT = nkb * 128
    NKC = (KT + 511) // 512      # kproj 512-col chunks
    nc = bass.Bass()
    xqT = nc.declare_dram_parameter("xqT", [D_MODEL, T], BF16, isOutput=False)
    xkT = nc.declare_dram_parameter("xkT", [D_MODEL, KT], BF16, isOutput=False)
    wqT = nc.declare_dram_parameter("wqT", [D_MODEL, NPAIR * 128], BF16, isOutput=False)
    wkT = nc.declare_dram_parameter("wkT", [D_MODEL, NPAIR * 128], BF16, isOutput=False)
    wvT = nc.declare_dram_parameter("wvT", [D_MODEL, CAT], BF16, isOutput=False)
    wtailT = nc.declare_dram_parameter("wtailT", [CAT, D_MODEL], BF16, isOutput=False)
    keep = nc.declare_dram_parameter("keep", [KT], F32, isOutput=False)
    ones8 = nc.declare_dram_parameter("ones8", [128, HPC], BF16, isOutput=False)
    sel4 = nc.declare_dram_parameter("sel4", [HPC, NPAIR * 128], F32R, isOutput=False)
    y = nc.declare_dram_parameter("y", [T, D_MODEL], F32, isOutput=True)

    with ExitStack() as ctx:
        tc = ctx.enter_context(tile.TileContext(nc))

        const = ctx.enter_context(tc.tile_pool(name="const", bufs=1))
        xq_pool = ctx.enter_context(tc.tile_pool(name="xq", bufs=1))
        xk_pool = ctx.enter_context(tc.tile_pool(name="xk", bufs=1))
        qk_pool = ctx.enter_context(tc.tile_pool(name="qk", bufs=1))
        va_pool = ctx.enter_context(tc.tile_pool(name="va", bufs=1))
        num_pool = ctx.enter_context(tc.tile_pool(name="num", bufs=1))
        w_pool = ctx.enter_context(tc.tile_pool(name="w", bufs=1))
        p_pool = ctx.enter_context(tc.tile_pool(name="p", bufs=1))
        stag_pool = ctx.enter_context(tc.tile_pool(name="stag", bufs=1))
        rb_pool = ctx.enter_context(tc.tile_pool(name="rb", bufs=1))
        ysb_pool = ctx.enter_context(tc.tile_pool(name="ysb", bufs=1))
        stp_pool = ctx.enter_context(tc.tile_pool(name="stp", bufs=1, space="PSUM"))
        avp_pool = ctx.enter_context(tc.tile_pool(name="avp", bufs=1, space="PSUM"))
        aux_pool = ctx.enter_context(tc.tile_pool(name="aux", bufs=1, space="PSUM"))

        # ---- consts
        sel4t = const.tile([HPC, NPAIR, 128], F32R, name="sel4")
        nc.sync.dma_start(
            out=sel4t, in_=sel4.rearrange("p (j c) -> p j c", c=128))
        keepc = const.tile([128, nkb], F32, name="keep")
        nc.sync.dma_start(out=keepc, in_=keep.rearrange("(j p) -> p j", p=128))
        ones8t = const.tile([128, HPC], BF16, name="ones8")
        nc.sync.dma_start(out=ones8t, in_=ones8[:, :])

        # ---- inputs (order = DMA priority: kproj/qproj first wave)
        wks = [w_pool.tile([128, KC, 128], BF16, tag=f"wk{j}", name=f"wk{j}")
               for j in range(NPAIR)]
        wqs = [w_pool.tile([128, KC, 128], BF16, tag=f"wq{j}", name=f"wq{j}")
               for j in range(NPAIR)]
        xks = [xk_pool.tile([128, KT], BF16, tag=f"xk{kc}", name=f"xk{kc}")
               for kc in range(KC)]
        xqs = [xq_pool.tile([128, T], BF16, tag=f"xq{kc}", name=f"xq{kc}")
               for kc in range(KC)]
        nc.sync.dma_start(
            out=wks[0],
            in_=wkT.rearrange("(kc p) c -> p kc c", p=128)[:, :, 0:128])
        for kc in range(KC):
            nc.sync.dma_start(out=xks[kc][:, 0:512],
                              in_=xkT[kc * 128:(kc + 1) * 128, 0:512])
        nc.sync.dma_start(
            out=wqs[0],
            in_=wqT.rearrange("(kc p) c -> p kc c", p=128)[:, :, 0:128])
        for kc in range(KC):
            nc.sync.dma_start(out=xqs[kc][:, 0:512],
                              in_=xqT[kc * 128:(kc + 1) * 128, 0:512])
        wvs = [w_pool.tile([128, CAT], BF16, tag=f"wv{kc}", name=f"wv{kc}")
               for kc in range(KC)]
        for kc in range(KC):
            nc.sync.dma_start(out=wvs[kc], in_=wvT[kc * 128:(kc + 1) * 128, :])
        if KT > 512:
            for kc in range(KC):
                nc.sync.dma_start(out=xks[kc][:, 512:KT],
                                  in_=xkT[kc * 128:(kc + 1) * 128, 512:KT])
        for j in range(1, NPAIR):
            nc.sync.dma_start(
                out=wks[j],
                in_=wkT.rearrange("(kc p) c -> p kc c", p=128)[
                    :, :, j * 128:(j + 1) * 128])
            nc.sync.dma_start(
                out=wqs[j],
                in_=wqT.rearrange("(kc p) c -> p kc c", p=128)[
                    :, :, j * 128:(j + 1) * 128])
        for kc in range(KC):
            nc.sync.dma_start(out=xqs[kc][:, 512:T],
                              in_=xqT[kc * 128:(kc + 1) * 128, 512:T])
        wts = [w_pool.tile([128, D_MODEL], BF16, tag=f"wt{c}", name=f"wt{c}")
               for c in range(CAT // 128)]
        for c in range(CAT // 128):
            nc.sync.dma_start(out=wts[c], in_=wtailT[c * 128:(c + 1) * 128, :])

        # ---- persistent intermediates
        qts = [qk_pool.tile([128, T], BF16, tag=f"qt{j}", name=f"qt{j}")
               for j in range(NPAIR)]
        kts = [qk_pool.tile([128, KT], BF16, tag=f"kt{j}", name=f"kt{j}")
               for j in range(NPAIR)]
        vaugs = [va_pool.tile([128, HPC, D_HEAD + 1], BF16, tag=f"va{t}",
                              name=f"va{t}") for t in range(nkb)]
        nums = [num_pool.tile([128, T], BF16, tag=f"nm{j}", name=f"nm{j}")
                for j in range(NPAIR)]
        # per-pair staged attn^T (+denominator row); written each round,
        # read by the deferred normalize during the NEXT round => bufs=2.
        def stag(j, h):
            return stag_pool.tile([D_HEAD + 1, 512], F32, tag=f"sg{j}{h}",
                                  name=f"sg{j}{h}", bufs=2)

        # ---- work quanta (each: one aux-psum accumulation group + evac)
        def vproj(tb):
            vp = aux_pool.tile([128, 512], F32, tag="aux", bufs=2)
            for kc in range(KC):
                nc.tensor.matmul(vp, xks[kc][:, tb * 128:(tb + 1) * 128],
                                 wvs[kc], start=(kc == 0), stop=(kc == KC - 1))
            va = vaugs[tb]
            nc.vector.tensor_scalar_mul(
                va[:, :, 0:D_HEAD],
                vp.rearrange("p (h d) -> p h d", h=HPC),
                keepc[:, tb:tb + 1])
            nc.vector.tensor_scalar_mul(
                va[:, :, D_HEAD:D_HEAD + 1].rearrange("p h o -> p (h o)"),
                ones8t,
                keepc[:, tb:tb + 1])

        def kproj_range(j, n0, n1):
            kp = aux_pool.tile([128, 512], F32, tag="aux", bufs=2)
            for kc in range(KC):
                nc.tensor.matmul(kp[:, 0:n1 - n0], wks[j][:, kc, :],
                                 xks[kc][:, n0:n1],
                                 start=(kc == 0), stop=(kc == KC - 1))
            nc.vector.tensor_copy(out=kts[j][:, n0:n1], in_=kp[:, 0:n1 - n0])

        def kproj(j, c):
            kproj_range(j, c * 512, min(c * 512 + 512, KT))

        def qproj(j, n):
            n0 = n * 512
            qp = aux_pool.tile([128, 512], F32, tag="aux", bufs=2)
            for kc in range(KC):
                nc.tensor.matmul(qp, wqs[j][:, kc, :], xqs[kc][:, n0:n0 + 512],
                                 start=(kc == 0), stop=(kc == KC - 1))
            nc.vector.tensor_copy(out=qts[j][:, n0:n0 + 512], in_=qp)

        def tailq(tb, n, evac_scalar=False):
            n0 = n * 512
            yp = aux_pool.tile([128, 512], F32, tag="aux", bufs=2)
            for c in range(NPAIR):
                nc.tensor.matmul(yp, nums[c][:, tb * 128:(tb + 1) * 128],
                                 wts[c][:, n0:n0 + 512],
                                 start=(c == 0), stop=(c == NPAIR - 1))
            y_sb = ysb_pool.tile([128, 512], F32, tag="ys", bufs=2)
            if evac_scalar:
                # drain only: the Scalar engine is idle there and this keeps
                # the aux-ring turnover off the DVE queue (which carries the
                # final reciprocal)
                nc.scalar.copy(out=y_sb, in_=yp)
            else:
                nc.vector.tensor_copy(out=y_sb, in_=yp)
            nc.sync.dma_start(out=y[tb * 128:(tb + 1) * 128, n0:n0 + 512],
                              in_=y_sb)

        # ---- deferred normalization for one qb round. The reciprocal (DVE,
        # ~6.5 ns/elem, batched to one [8,512] op/round) is emitted at round
        # end so it runs during the next round's attention; the broadcast +
        # apply quanta are consumed from the deque well after it finished.
        def make_norm_quanta(qb, rball, round_stags):
            q0 = qb * 512

            def recipq():
                rtmp = rb_pool.tile([HPC, 512], F32, tag="rtmp", bufs=2)
                nc.vector.reciprocal(out=rtmp, in_=rball)
                rrec = rb_pool.tile([HPC, 512], F32R, tag="rrec", bufs=2)
                nc.vector.tensor_copy(out=rrec, in_=rtmp)
                rrec_ref[0] = rrec

            rrec_ref = [None]
            quanta = [recipq]

            def normj(j):
                rbp = aux_pool.tile([128, 512], F32, tag="aux", bufs=2)
                nc.tensor.matmul(rbp, sel4t[:, j, :], rrec_ref[0],
                                 start=True, stop=True)
                nc.vector.tensor_tensor(
                    out=nums[j][0:64, q0:q0 + 512],
                    in0=round_stags[j][0][0:64, :],
                    in1=rbp[0:64, :], op=mybir.AluOpType.mult)
                nc.vector.tensor_tensor(
                    out=nums[j][64:128, q0:q0 + 512],
                    in0=round_stags[j][1][0:64, :],
                    in1=rbp[64:128, :], op=mybir.AluOpType.mult)

            quanta.extend(lambda jj=j: normj(jj) for j in range(NPAIR))
            return quanta

        # general work deque (FIFO; consumed one per slot)
        deque = []
        for n in range(1, NQB):
            for j in range(NPAIR):
                deque.append((lambda jj=j, nn=n: qproj(jj, nn)))

        def pop_deque():
            if deque:
                deque.pop(0)()

        # direct normalize for one pair (used in the hand-scheduled drain)
        def normd(j, q0, selap, rrec_ap, sgA, sgB):
            rbp = aux_pool.tile([128, 512], F32, tag="aux", bufs=2)
            nc.tensor.matmul(rbp, selap, rrec_ap, start=True, stop=True)
            nc.vector.tensor_tensor(
                out=nums[j][0:64, q0:q0 + 512], in0=sgA[0:64, :],
                in1=rbp[0:64, :], op=mybir.AluOpType.mult)
            nc.vector.tensor_tensor(
                out=nums[j][64:128, q0:q0 + 512], in0=sgB[0:64, :],
                in1=rbp[64:128, :], op=mybir.AluOpType.mult)

        # ---- attention unit
        def unit(u, j, qb, rball, rballB=None):
            q0 = qb * 512
            h0, h1 = 2 * j, 2 * j + 1
            avpA = avp_pool.tile([D_HEAD + 1, 512], F32, tag="avpA",
                                 name="avpA")
            avpB = avp_pool.tile([D_HEAD + 1, 512], F32, tag="avpB",
                                 name="avpB")
            ps = {}

            def st_exp(kb):
                stp = stp_pool.tile([128, 2, 512], F32, tag="stp", bufs=2)
                nc.tensor.matmul(
                    stp[:, 0, :], kts[j][0:64, kb * 128:(kb + 1) * 128],
                    qts[j][0:64, q0:q0 + 512], start=True, stop=True,
                    tile_position=(0, 0))
                nc.tensor.matmul(
                    stp[:, 1, :], kts[j][64:128, kb * 128:(kb + 1) * 128],
                    qts[j][64:128, q0:q0 + 512], start=True, stop=True,
                    tile_position=(64, 0))
                p = p_pool.tile([128, 2, 512], BF16, tag="p", bufs=3)
                nc.scalar.activation(
                    out=p, in_=stp, func=mybir.ActivationFunctionType.Exp,
                    scale=0.125)
                ps[kb] = p

            def av(kb):
                p = ps.pop(kb)
                nc.tensor.matmul(avpA, vaugs[kb][:, h0, :], p[:, 0, :],
                                 start=(kb == 0), stop=(kb == nkb - 1))
                nc.tensor.matmul(avpB, vaugs[kb][:, h1, :], p[:, 1, :],
                                 start=(kb == 0), stop=(kb == nkb - 1))

            st_exp(0)
            for kb in range(1, nkb):
                # scheduled filler work for this slot
                if u == 0:
                    if kb == 1:
                        vproj(0)
                        vproj(1)
                        vproj(2)
                    elif kb <= NKC:
                        kproj(0, kb - 1)
                        if kb + 1 < nkb:
                            vproj(kb + 1)
                    elif kb + 1 < nkb:
                        vproj(kb + 1)
                elif u in (1, 2, 3):
                    if 1 <= kb < NKC:
                        kproj(j, kb)
                    elif kb >= 3:
                        pop_deque()
                else:
                    pop_deque()
                st_exp(kb)
                av(kb - 1)
            av(nkb - 1)

            # stage attn^T + denominator row; gather D rows for the round
            sgA, sgB = stag(j, 0), stag(j, 1)
            nc.vector.tensor_copy(out=sgA, in_=avpA)
            nc.vector.tensor_copy(out=sgB, in_=avpB)
            if rballB is not None:
                # last pair of the last round: its own 2-row gather so the
                # drain reciprocal is partition-base-0 (32-align rule)
                nc.sync.dma_start(out=rballB[0:1, :],
                                  in_=sgA[D_HEAD:D_HEAD + 1, :])
                nc.sync.dma_start(out=rballB[1:2, :],
                                  in_=sgB[D_HEAD:D_HEAD + 1, :])
            else:
                nc.sync.dma_start(out=rball[h0:h0 + 1, :],
                                  in_=sgA[D_HEAD:D_HEAD + 1, :])
                nc.sync.dma_start(out=rball[h1:h1 + 1, :],
                                  in_=sgB[D_HEAD:D_HEAD + 1, :])
            return sgA, sgB

        # ---- prologue: minimum to start unit(p0, qb0)
        kproj(0, 0)
        qproj(0, 0)

        u = 0
        for qb in range(NQB):
            last = qb == NQB - 1
            rball = rb_pool.tile([HPC, 512], F32, tag="rball", bufs=2)
            rballB = (rb_pool.tile([2, 512], F32, tag="rballB", name="rballB")
                      if last else None)
            rrecA = None
            round_stags = []
            for j in range(NPAIR):
                if u in (1, 2, 3):
                    kproj(j, 0)
                    qproj(j, 0)
                round_stags.append(
                    unit(u, j, qb, rball,
                         rballB if (last and j == NPAIR - 1) else None))
                u += 1
                if last and j == NPAIR - 2:
                    # pairs 0-2 denominators are complete: their reciprocal
                    # runs on the idle DVE under the last unit's attention,
                    # leaving only a [2,512] recip on the drain path.
                    rtmpA = rb_pool.tile([6, 512], F32, tag="rtA", name="rtA")
                    nc.vector.reciprocal(out=rtmpA, in_=rball[0:6, :])
                    rrecA = rb_pool.tile([6, 512], F32R, tag="rrA", name="rrA")
                    nc.vector.tensor_copy(out=rrecA, in_=rtmpA)
            if not last:
                # tail(qb-1) first (dense, dependency-free at round start),
                # then recip+norm(qb): the reciprocal lands ~11us into the
                # next round where the DVE is idle, before its consumers.
                if qb >= 1:
                    for tb in range((qb - 1) * 4, qb * 4):
                        for n in range(2):
                            deque.append((lambda t=tb, nn=n: tailq(t, nn)))
                deque.extend(make_norm_quanta(qb, rball, round_stags))
        # ---- hand-scheduled drain for the last round
        qb = NQB - 1
        q0 = qb * 512
        # pair-3 reciprocal first in the drain's DVE stream: tail evacs go
        # through the Scalar engine, so nothing queues behind it
        rtmpB = rb_pool.tile([2, 512], F32, tag="rtB", name="rtB")
        nc.vector.reciprocal(out=rtmpB, in_=rballB)
        rrecB = rb_pool.tile([2, 512], F32R, tag="rrB", name="rrB")
        nc.vector.tensor_copy(out=rrecB, in_=rtmpB)
        for tb in range((qb - 1) * 4, qb * 4):
            for n in range(2):
                deque.append((lambda t=tb, nn=n: tailq(t, nn, True)))
        pop_deque()
        pop_deque()
        pop_deque()
        for j in range(NPAIR - 1):
            normd(j, q0, sel4t[0:6, j, :], rrecA,
                  round_stags[j][0], round_stags[j][1])
        # the remaining tail(qb-1) work covers the pair-3 reciprocal latency
        for _ in range(5):
            pop_deque()
        # pair 3: selector rows 0-1 of pair 0 have exactly the 2-row pattern
        normd(NPAIR - 1, q0, sel4t[0:2, 0, :], rrecB,
              round_stags[NPAIR - 1][0], round_stags[NPAIR - 1][1])
        for tb in range(qb * 4, (qb + 1) * 4):
            for n in range(2):
                deque.append((lambda t=tb, nn=n: tailq(t, nn, True)))
        while deque:
            pop_deque()

    if split_waits:
        split_excess_waits(nc)
    return nc


_NC_CACHE = {}


def _get_nc(nkb):
    if nkb not in _NC_CACHE:
        _NC_CACHE[nkb] = build_nc(nkb)
    return _NC_CACHE[nkb]


def make_in_maps(x, mask, w_qkv, w_tail):
    """Shard full inputs into 8 per-core input maps (with key gather)."""
    x = np.asarray(x, dtype=np.float32)
    mask = np.asarray(mask, dtype=np.int32)
    w_qkv = np.asarray(w_qkv, dtype=np.float32)
    w_tail = np.asarray(w_tail, dtype=np.float32)

    # per-batch kept-key gather
    idxs = [np.nonzero(mask[b] != 0)[0] for b in range(BN)]
    nkb = max(4, max((len(ix) + 127) // 128 for ix in idxs))
    KT = nkb * 128

    xk_all, keep_all = [], []
    for b in range(BN):
        ix = idxs[b]
        m = len(ix)
        xk = np.zeros((KT, D_MODEL), dtype=np.float32)
        xk[:m] = x[b][ix]
        kp = np.zeros((KT,), dtype=np.float32)
        kp[:m] = 1.0
        xk_all.append(xk)
        keep_all.append(kp)

    w3 = w_qkv.reshape(N_HEAD, 3, D_HEAD, D_MODEL)  # [head, q|k|v, d, dm]
    # sel4[:, j*128:(j+1)*128]: broadcast selector for pair j
    selv = np.zeros((HPC, NPAIR * 128), np.float32)
    for j in range(NPAIR):
        selv[2 * j, j * 128:j * 128 + 64] = 1.0
        selv[2 * j + 1, j * 128 + 64:(j + 1) * 128] = 1.0

    in_maps = []
    for c in range(8):
        b, hg = c // 2, c % 2
        heads = [hg * HPC + i for i in range(HPC)]
        wq = np.concatenate([w3[h, 0] for h in heads], axis=0)  # [512, 1024]
        wk = np.concatenate([w3[h, 1] for h in heads], axis=0)
        wv = np.concatenate([w3[h, 2] for h in heads], axis=0)
        wt = w_tail[:, hg * CAT:(hg + 1) * CAT]  # [1024, 512]
        in_maps.append({
            "xqT": np.ascontiguousarray(x[b].T).astype(BF),
            "xkT": np.ascontiguousarray(xk_all[b].T).astype(BF),
            "wqT": np.ascontiguousarray(wq.T).astype(BF),
            "wkT": np.ascontiguousarray(wk.T).astype(BF),
            "wvT": np.ascontiguousarray(wv.T).astype(BF),
            "wtailT": np.ascontiguousarray(wt.T).astype(BF),
            "keep": keep_all[b],
            "ones8": np.ones((128, HPC), dtype=BF),
            "sel4": selv,
        })
    return in_maps, nkb


def kernel(x, mask, w_qkv, w_tail, b_tail):
    in_maps, nkb = make_in_maps(x, mask, w_qkv, w_tail)
    nc = _get_nc(nkb)
    last_err = None
    for _attempt in range(3):
        try:
            res = run_bass_kernel_spmd(nc, in_maps, list(range(8))).results
            break
        except Exception as e:  # transient device/runtime errors: retry
            last_err = e
            _time.sleep(3.0)
    else:
        raise last_err
    out = np.empty((BN, T, D_MODEL), dtype=np.float32)
    b_tail = np.asarray(b_tail, dtype=np.float32)
    for b in range(BN):
        out[b] = res[2 * b]["y"] + res[2 * b + 1]["y"] + b_tail
    return out
